# revision 1
# baseline (speedup 1.0000x reference)
"""Causal self-attention (B=2, T=2048, C=1024, H=16) on 8 TRN2 NeuronCores.

Sharding: core = b*4 + hg  (data parallel over batch, tensor parallel over
4 head-groups of 4 heads). Each core computes its head-group's attention and
a partial output projection; the host sums the 4 partials per batch and adds
b_proj.

Per-core device program (v2 - overlap-oriented):
  - x, Wqk, Wv are loaded in bf16 (halves the input DMA); q/k are kept bf16;
    v, p, y are float32r (fp22) so the o/proj matmuls run at full PE rate
    with ~1e-4 precision; all PSUM accumulation is fp32.
  - v_aug carries a per-head ones column placed so the o-matmul accumulates
    the softmax denominator D on PSUM partition {64,0,96,32}[h] while the
    head's v columns land exactly on its yT rows.
  - Attention runs per (head, 1024-wide query chunk) with causality at
    128-block granularity; ACT computes exp(s/32) straight out of PSUM;
    triangular masks are multiplied on the (otherwise idle) GPSIMD engine.
  - Separate PSUM pools for qkv (Q), scores (A), o-accumulation (B) let the
    scheduler overlap attention of head-pair A with qkv of head-pair B.
  - 1/D rows are broadcast across partitions by a DRAM round-trip DMA with a
    0-stride source AP; each head pair is normalized as soon as it finishes.
"""

import math

import numpy as np

import concourse.bass as bass
import concourse.bacc as bacc
import concourse.mybir as mybir
from concourse import tile
from concourse.bass_utils import run_bass_kernel_spmd

B, T, C, H = 2, 2048, 1024, 16
HD = C // H   # 64
HPG = 4       # heads per group
NG = 4        # head groups
NCORES = 8

F32 = mybir.dt.float32
F32R = mybir.dt.float32r
BF16 = mybir.dt.bfloat16
F16 = mybir.dt.float16
AF = mybir.ActivationFunctionType
SCALE = 1.0 / math.sqrt(C)  # 1/32

# Per-head layout of the v_aug stationary block: (col offset, width,
# v-column offset within block, ones-column offset within block).
# v columns sit at PSUM rows (h%2)*64..+64; ones column on a 32-aligned row.
V_BLK = [
    (0, 65, 0, 64),      # h0: v@0-63,  D@64
    (65, 128, 64, 0),    # h1: v@64-127, D@0
    (193, 97, 0, 96),    # h2: v@0-63,  D@96
    (290, 128, 64, 32),  # h3: v@64-127, D@32
]
VW = 418  # total v_aug width
DROW = [64, 0, 96, 32]  # PSUM partition of D per head


def _pieces(L):
    """Bank-aligned (offset, width) pieces covering cols [L, 1024) of a
    1024-wide fp32 PSUM span (bank boundary at 512 fp32)."""
    assert 0 <= L < 1024 and L % 128 == 0
    if L < 512:
        return [(L, 512 - L), (512, 512)]
    return [(L, 1024 - L)]


def build_program(reps=1):
    nc = bacc.Bacc()

    xT = nc.dram_tensor("xT", [C, T], F16, kind="ExternalInput")
    wqk = nc.dram_tensor("wqk", [C, 512], F16, kind="ExternalInput")
    bqk = nc.dram_tensor("bqk", [128, 4], F32, kind="ExternalInput")
    wv = nc.dram_tensor("wv", [C, VW], F16, kind="ExternalInput")
    bv = nc.dram_tensor("bv", [1, VW], F16, kind="ExternalInput")
    wp = nc.dram_tensor("wp", [256, 1024], F32R, kind="ExternalInput")
    mask = nc.dram_tensor("mask", [128, 128], F32, kind="ExternalInput")
    ones = nc.dram_tensor("ones", [1, 128], F16, kind="ExternalInput")
    onesf = nc.dram_tensor("onesf", [128, 128], F32R, kind="ExternalInput")
    out = nc.dram_tensor("out", [T, C], F32, kind="ExternalOutput")

    with tile.TileContext(nc) as tc:
        with (
            tc.tile_pool(name="big", bufs=8) as big_pool,
            tc.tile_pool(name="pp", bufs=4) as p_pool,
            tc.tile_pool(name="osb", bufs=3) as o_pool,
            tc.tile_pool(name="wqk", bufs=8) as wqk_pool,
            tc.tile_pool(name="wv", bufs=8) as wv_pool,
            tc.tile_pool(name="qkT", bufs=4) as qkT_pool,
            tc.tile_pool(name="vsb", bufs=16) as v_pool,
            tc.tile_pool(name="yT", bufs=2) as yT_pool,
            tc.tile_pool(name="wp", bufs=2) as wp_pool,
            tc.tile_pool(name="consts", bufs=1) as c_pool,
            tc.tile_pool(name="psQ", bufs=2, space="PSUM") as psQ,
            tc.tile_pool(name="psA", bufs=2, space="PSUM") as psA,
            tc.tile_pool(name="psB", bufs=2, space="PSUM") as psB,
        ):
          for rep in range(reps):
            # ---- loads (wqk/xt interleaved so compute starts early) ----
            d128 = c_pool.tile([128, T], F32, tag="d128")
            nc.gpsimd.memset(d128[:], 1.0)
            r128 = c_pool.tile([128, T], F32R, tag="r128")
            xt_sb, wqk_sb, wv_sb = [], [], []
            # spread load DMAs over four DGE paths so issue doesn't serialize
            for ct in range(8):
                w_ = wqk_pool.tile([128, 512], F16, tag="wqk", name=f"wqk{ct}")
                weng = nc.scalar if ct % 2 == 0 else nc.sync
                weng.dma_start(w_[:], wqk[ct * 128:(ct + 1) * 128, :])
                wqk_sb.append(w_)
                t_ = big_pool.tile([128, T], F16, tag="big", name=f"xt{ct}")
                eng = nc.sync if ct % 2 == 0 else nc.scalar
                eng.dma_start(t_[:], xT[ct * 128:(ct + 1) * 128, :])
                xt_sb.append(t_)
            for ct in range(8):
                t_ = wv_pool.tile([128, VW], F16, tag="wv", name=f"wv{ct}")
                nc.gpsimd.dma_start(t_[:], wv[ct * 128:(ct + 1) * 128, :])
                wv_sb.append(t_)
            bqk_sb = c_pool.tile([128, 4], F32, tag="bqk")
            nc.gpsimd.dma_start(bqk_sb[:], bqk[:])
            bv_sb = c_pool.tile([1, VW], F16, tag="bv")
            nc.gpsimd.dma_start(bv_sb[:], bv[:])
            mask_sb = c_pool.tile([128, 128], F32, tag="mask")
            nc.gpsimd.dma_start(mask_sb[:], mask[:])
            ones_sb = c_pool.tile([1, 128], F16, tag="ones")
            nc.gpsimd.dma_start(ones_sb[:], ones[:])
            onesf_sb = c_pool.tile([128, 128], F32R, tag="onesf")
            nc.gpsimd.dma_start(onesf_sb[:], onesf[:])
            wp_sb = []
            for mt in range(2):
                t_ = wp_pool.tile([128, 1024], F32R, tag="wp", name=f"wp{mt}")
                nc.sync.dma_start(t_[:], wp[mt * 128:(mt + 1) * 128, :])
                wp_sb.append(t_)

            qkT_sb = [
                qkT_pool.tile([128, T], F16, tag="qkT", name=f"qkT{j}")
                for j in range(4)
            ]
            yT_sb = [
                yT_pool.tile([128, T], F32R, tag="yT", name=f"yT{m}")
                for m in range(2)
            ]

            def emit_qk(jts):
                for jt in jts:
                    for ch in range(4):
                        ps = psQ.tile([128, 512], F32, tag="Q", name="qk_ps")
                        for ct in range(8):
                            nc.tensor.matmul(
                                ps[:, 0:512],
                                wqk_sb[ct][:, jt * 128:(jt + 1) * 128],
                                xt_sb[ct][:, ch * 512:(ch + 1) * 512],
                                start=(ct == 0),
                                stop=(ct == 7),
                            )
                        nc.vector.tensor_scalar_add(
                            qkT_sb[jt][:, ch * 512:(ch + 1) * 512],
                            ps[:, 0:512],
                            bqk_sb[:, jt:jt + 1],
                        )

            def emit_v():
                v_sb = []
                for tt in range(16):
                    ps = psQ.tile([128, 512], F32, tag="Q", name="v_ps")
                    for ct in range(8):
                        nc.tensor.matmul(
                            ps[:, 0:VW],
                            xt_sb[ct][:, tt * 128:(tt + 1) * 128],
                            wv_sb[ct][:, 0:VW],
                            start=(ct == 0),
                            stop=False,
                        )
                    nc.tensor.matmul(
                        ps[:, 0:VW],
                        ones_sb[0:1, 0:128],
                        bv_sb[0:1, 0:VW],
                        start=False,
                        stop=True,
                    )
                    t_ = v_pool.tile([128, VW], F32R, tag="v", name=f"v{tt}")
                    nc.scalar.copy(t_[:], ps[:, 0:VW])
                    v_sb.append(t_)
                return v_sb

            def emit_attention(h, v_sb):
                qrow = (h % 2) * 64
                q_tile = qkT_sb[h // 2]
                k_tile = qkT_sb[2 + h // 2]
                blk_off, blk_w, v_off, one_off = V_BLK[h]
                dr = DROW[h]
                for ic in range(2):
                    i0 = ic * 1024
                    o_a = psB.tile([128, 512], F32, tag="B", name="o_a")
                    o_b = psB.tile([128, 512], F32, tag="B", name="o_b")
                    njt = 8 * (ic + 1)
                    for jt in range(njt):
                        L = max(0, jt * 128 - i0)
                        s_ps = psA.tile([128, 1024], F32, tag="A", name="s_ps")
                        for off, w in _pieces(L):
                            nc.tensor.matmul(
                                s_ps[:, off:off + w],
                                k_tile[qrow:qrow + 64, jt * 128:(jt + 1) * 128],
                                q_tile[qrow:qrow + 64, i0 + off:i0 + off + w],
                                start=True,
                                stop=True,
                            )
                        p_sb = p_pool.tile([128, 1024], F32R, tag="p",
                                           name="p_sb")
                        nc.scalar.activation(
                            p_sb[:, L:1024], s_ps[:, L:1024], AF.Exp, scale=SCALE
                        )
                        if jt * 128 >= i0:  # diagonal block: triangular mask
                            nc.gpsimd.tensor_mul(
                                p_sb[:, L:L + 128], p_sb[:, L:L + 128], mask_sb[:]
                            )
                        vap = v_sb[jt][:, blk_off:blk_off + blk_w]
                        for off, w in _pieces(L):
                            tgt = o_a if off < 512 else o_b
                            toff = off if off < 512 else off - 512
                            nc.tensor.matmul(
                                tgt[0:blk_w, toff:toff + w],
                                vap,
                                p_sb[:, off:off + w],
                                start=(jt == 0),
                                stop=(jt == njt - 1),
                                skip_group_check=True,
                            )
                    for half, o_x in ((0, o_a), (1, o_b)):
                        c0 = i0 + half * 512
                        nc.vector.tensor_copy(
                            yT_sb[h // 2][qrow:qrow + 64, c0:c0 + 512],
                            o_x[v_off:v_off + 64, :],
                        )
                        nc.vector.tensor_copy(
                            d128[dr:dr + 1, c0:c0 + 512],
                            o_x[one_off:one_off + 1, :],
                        )

            def emit_normalize(mt):
                ha, hb = 2 * mt, 2 * mt + 1
                pa, pb = DROW[ha], DROW[hb]
                hi = max(pa, pb)
                with nc.allow_low_precision(reason="1/D fp32r feeds fp32r mm"):
                    nc.vector.reciprocal(
                        r128[0:hi + 1, :], d128[0:hi + 1, :]
                    )
                # K=1 ones-matmuls broadcast each head's 1/D row to all 128
                # PSUM partitions; the scale muls then read the matching half.
                for ic in range(2):
                    for nch in range(2):
                        sl = slice(ic * 1024 + nch * 512,
                                   ic * 1024 + (nch + 1) * 512)
                        rba = psQ.tile([128, 512], F32, tag="Q", name="rba")
                        nc.tensor.matmul(
                            rba[:, 0:512],
                            onesf_sb[pa:pa + 1, 0:128],
                            r128[pa:pa + 1, sl],
                            start=True,
                            stop=True,
                            tile_position=(pa, 0),
                        )
                        rbb = psQ.tile([128, 512], F32, tag="Q", name="rbb")
                        nc.tensor.matmul(
                            rbb[:, 0:512],
                            onesf_sb[pb:pb + 1, 0:128],
                            r128[pb:pb + 1, sl],
                            start=True,
                            stop=True,
                            tile_position=(pb, 0),
                        )
                        nc.vector.tensor_mul(
                            yT_sb[mt][0:64, sl], yT_sb[mt][0:64, sl],
                            rba[0:64, 0:512],
                        )
                        nc.vector.tensor_mul(
                            yT_sb[mt][64:128, sl], yT_sb[mt][64:128, sl],
                            rbb[64:128, 0:512],
                        )

            emit_qk((0, 2))        # q and k tiles for head pair A (h0, h1)
            v_sb = emit_v()
            emit_qk((1, 3))        # head pair B (h2, h3)
            emit_attention(0, v_sb)
            emit_attention(1, v_sb)
            emit_normalize(0)
            emit_attention(2, v_sb)
            emit_attention(3, v_sb)
            emit_normalize(1)

            # ---- projection + output ----
            for tt in range(16):
                ps = psA.tile([128, 1024], F32, tag="A", name="pj_ps")
                for mt in range(2):
                    for nch in range(2):
                        nc.tensor.matmul(
                            ps[:, nch * 512:(nch + 1) * 512],
                            yT_sb[mt][:, tt * 128:(tt + 1) * 128],
                            wp_sb[mt][:, nch * 512:(nch + 1) * 512],
                            start=(mt == 0),
                            stop=(mt == 1),
                        )
                o_sb = o_pool.tile([128, 1024], F32, tag="o", name="o_sb")
                if tt % 2 == 0:
                    nc.vector.tensor_copy(o_sb[:], ps[:, 0:1024])
                else:
                    nc.scalar.copy(o_sb[:], ps[:, 0:1024])
                nc.sync.dma_start(out[tt * 128:(tt + 1) * 128, :], o_sb[:])

    if not nc.is_finalized():
        nc.finalize()
    return nc


def host_prep(x, W_attn, b_attn, W_proj):
    import ml_dtypes
    bf = np.float16
    x = np.ascontiguousarray(np.asarray(x, np.float32))
    W_attn = np.ascontiguousarray(np.asarray(W_attn, np.float32))
    b_attn = np.ascontiguousarray(np.asarray(b_attn, np.float32))
    W_proj = np.ascontiguousarray(np.asarray(W_proj, np.float32))
    mask = np.triu(np.ones((128, 128), np.float32))
    ones = np.ones((1, 128), bf)
    onesf = np.ones((128, 128), np.float32)
    per_group = []
    for hg in range(NG):
        heads = [hg * HPG + i for i in range(HPG)]
        wq = np.concatenate([W_attn[:, h * HD:(h + 1) * HD] for h in heads], axis=1)
        wk = np.concatenate(
            [W_attn[:, C + h * HD:C + (h + 1) * HD] for h in heads], axis=1
        )
        wqk_ = np.ascontiguousarray(np.concatenate([wq, wk], axis=1).astype(bf))
        bq = np.concatenate([b_attn[h * HD:(h + 1) * HD] for h in heads])
        bk = np.concatenate([b_attn[C + h * HD:C + (h + 1) * HD] for h in heads])
        bqk_ = np.ascontiguousarray(np.concatenate([bq, bk]).reshape(4, 128).T)
        wv_ = np.zeros((C, VW), np.float32)
        bv_ = np.zeros((1, VW), np.float32)
        for i, h in enumerate(heads):
            blk_off, blk_w, v_off, one_off = V_BLK[i]
            wv_[:, blk_off + v_off:blk_off + v_off + 64] = \
                W_attn[:, 2 * C + h * HD:2 * C + (h + 1) * HD]
            bv_[0, blk_off + v_off:blk_off + v_off + 64] = \
                b_attn[2 * C + h * HD:2 * C + (h + 1) * HD]
            bv_[0, blk_off + one_off] = 1.0
        wp_ = np.ascontiguousarray(
            np.concatenate([W_proj[h * HD:(h + 1) * HD, :] for h in heads], axis=0)
        )
        per_group.append((wqk_, bqk_, wv_.astype(bf), bv_.astype(bf), wp_))
    in_maps = []
    for b in range(B):
        xT_b = np.ascontiguousarray(x[b].T.astype(bf))
        for hg in range(NG):
            wqk_, bqk_, wv_, bv_, wp_ = per_group[hg]
            in_maps.append(
                dict(xT=xT_b, wqk=wqk_, bqk=bqk_, wv=wv_, bv=bv_, wp=wp_,
                     mask=mask, ones=ones, onesf=onesf)
            )
    return in_maps


_prog_cache = {}


def _get_program():
    if "nc" not in _prog_cache:
        _prog_cache["nc"] = build_program()
    return _prog_cache["nc"]


def run_cores(in_maps, trace=False, **kw):
    return run_bass_kernel_spmd(
        _get_program(), in_maps, list(range(NCORES)), trace=trace, **kw
    )


def kernel(x, W_attn, b_attn, W_proj, b_proj):
    in_maps = host_prep(x, W_attn, b_attn, W_proj)
    br = run_cores(in_maps)
    b_proj = np.asarray(b_proj, np.float32)
    y = np.zeros((B, T, C), np.float32)
    for b in range(B):
        acc = np.zeros((T, C), np.float32)
        for hg in range(NG):
            acc += np.asarray(br.results[b * NG + hg]["out"])
        y[b] = acc + b_proj[None, :]
    return y

